# revision 1
# baseline (speedup 1.0000x reference)
"""Trainium2 Bass kernel: RoPE causal attention (B=1,S=2048,D=4096,H=32).

Tensor-parallel over heads on 8 NeuronCores: core c owns heads [4c,4c+4).
Per core: q/k/v projections of its 4 heads (bf16 matmuls, f32 accum), RoPE,
causal flash-ish attention, and the wo matmul against its 512-column slice
of wo -> a full (2048,4096) partial output. Host sums the 8 partials.
"""

import math
import numpy as np

import concourse.bass as bass
import concourse.mybir as mybir
import concourse.tile as tile
from concourse import bacc
from concourse.bass import ts, ds
from concourse.bass_utils import run_bass_kernel_spmd
from concourse.kernels.tile_matmul import matmul_tile_kernel
from concourse.masks import make_identity

B, S, D, H, HD = 1, 2048, 4096, 32, 128
NCORES = 8
HL = H // NCORES          # 4 heads per core
DL = HL * HD              # 512 local head dims
NT = S // 128             # 16 seq tiles
KH = HD // 2              # 64 rope pairs
SCALE = 1.0 / math.sqrt(HD)
F32 = mybir.dt.float32
BF16 = mybir.dt.bfloat16

_CACHE = {}


def _build():
    nc = bacc.Bacc(None, target_bir_lowering=False, debug=False)
    x_t = nc.dram_tensor("x", [S, D], F32, kind="ExternalInput")
    fra_t = nc.dram_tensor("fra", [S, KH], F32, kind="ExternalInput")
    frb_t = nc.dram_tensor("frb", [S, KH], F32, kind="ExternalInput")
    wq_t = nc.dram_tensor("wq", [DL, D], F32, kind="ExternalInput")
    wk_t = nc.dram_tensor("wk", [DL, D], F32, kind="ExternalInput")
    wv_t = nc.dram_tensor("wv", [DL, D], F32, kind="ExternalInput")
    wo_t = nc.dram_tensor("wo", [D, DL], F32, kind="ExternalInput")
    cm_t = nc.dram_tensor("cmask", [128, 128], F32, kind="ExternalInput")
    y_t = nc.dram_tensor("y", [S, D], BF16, kind="ExternalOutput")

    with tile.TileContext(nc) as tc:
        with tc.tile_pool(name="dram", bufs=1, space="DRAM") as dram:
            x16 = dram.tile([S, D], BF16)
            wq16 = dram.tile([DL, D], BF16)
            wk16 = dram.tile([DL, D], BF16)
            wv16 = dram.tile([DL, D], BF16)
            wo16 = dram.tile([D, DL], BF16)
            q16 = dram.tile([S, DL], BF16)
            k16 = dram.tile([S, DL], BF16)
            v16 = dram.tile([S, DL], BF16)
            att16 = dram.tile([DL, S], BF16)  # transposed attention output

            # ---- stage 0: cast inputs f32 -> bf16 via SWDGE cast-DMA ----
            with tc.tile_pool(name="cast", bufs=8) as cp:
                def cast2d(src_ap, dst_tile, rows, cols):
                    for r in range(0, rows, 128):
                        t = cp.tile([128, cols], BF16, tag="cast")
                        nc.gpsimd.dma_start(out=t[:], in_=src_ap[r:r + 128, :])
                        nc.sync.dma_start(out=dst_tile[r:r + 128, :], in_=t[:])
                cast2d(x_t, x16, S, D)
                cast2d(wq_t, wq16, DL, D)
                cast2d(wk_t, wk16, DL, D)
                cast2d(wv_t, wv16, DL, D)
                cast2d(wo_t, wo16, D, DL)

            # ---- stage 1: projections q,k,v = x @ w.T ----
            for w16, o16 in ((wq16, q16), (wk16, k16), (wv16, v16)):
                matmul_tile_kernel(
                    tc, x16[:], w16[:], o16[:],
                    transpose_kxm=True, transpose_kxn=True,
                )

            # ---- stages 2-3: rope + causal attention ----
            with (
                tc.tile_pool(name="const", bufs=1) as const,
                tc.tile_pool(name="persist", bufs=1) as pers,
                tc.tile_pool(name="work", bufs=4) as work,
                tc.tile_pool(name="strips", bufs=3) as strips,
                tc.tile_pool(name="stats", bufs=6) as stats,
                tc.tile_pool(name="pst", bufs=2, space="PSUM") as pst,
                tc.tile_pool(name="pso", bufs=2, space="PSUM") as pso,
            ):
                ident = const.tile([128, 128], BF16)
                make_identity(nc, ident)
                cmask = const.tile([128, 128], F32)
                nc.sync.dma_start(out=cmask[:], in_=cm_t[:, :])

                qT = pers.tile([128, HL, S], BF16)   # [hd, h, s]
                kT = pers.tile([128, HL, S], BF16)
                vS = pers.tile([128, NT, DL], BF16)  # [s%128, s//128, dl]
                cosr = pers.tile([128, NT, HL, KH], F32)
                sinr = pers.tile([128, NT, HL, KH], F32)

                # cos/sin replicated per head. ACT Sin is only valid on
                # [-pi, pi]; host passes fra = wrap(freqs), frb = wrap(freqs+pi/2)
                # so sin(freqs)=Sin(fra), cos(freqs)=Sin(frb).
                for t in range(NT):
                    fra = work.tile([128, KH], F32, tag="fra")
                    frb = work.tile([128, KH], F32, tag="frb")
                    nc.sync.dma_start(out=fra[:], in_=fra_t[t * 128:(t + 1) * 128, :])
                    nc.sync.dma_start(out=frb[:], in_=frb_t[t * 128:(t + 1) * 128, :])
                    for h in range(HL):
                        nc.scalar.activation(sinr[:, t, h], fra[:], mybir.ActivationFunctionType.Sin)
                        nc.scalar.activation(cosr[:, t, h], frb[:], mybir.ActivationFunctionType.Sin)

                # v load
                for t in range(NT):
                    nc.sync.dma_start(out=vS[:, t], in_=v16[t * 128:(t + 1) * 128, :])

                # rope(q), rope(k), then per-128 transpose into qT/kT
                for src16, dstT in ((q16, qT), (k16, kT)):
                    for t in range(NT):
                        raw = work.tile([128, HL, KH, 2], BF16, tag="raw")
                        rot = work.tile([128, HL, KH, 2], BF16, tag="rot")
                        tmp = work.tile([128, HL, KH, 2], F32, tag="tmp")
                        nc.sync.dma_start(out=raw[:], in_=src16[t * 128:(t + 1) * 128, :])
                        t0, t1 = raw[:, :, :, 0], raw[:, :, :, 1]
                        c_, s_ = cosr[:, t], sinr[:, t]
                        # o0 = t0*c - t1*s ; o1 = t0*s + t1*c
                        nc.vector.tensor_tensor(out=tmp[:, :, :, 0], in0=t0, in1=c_, op=mybir.AluOpType.mult)
                        nc.vector.tensor_tensor(out=tmp[:, :, :, 1], in0=t1, in1=s_, op=mybir.AluOpType.mult)
                        nc.vector.tensor_tensor(out=rot[:, :, :, 0], in0=tmp[:, :, :, 0], in1=tmp[:, :, :, 1], op=mybir.AluOpType.subtract)
                        nc.vector.tensor_tensor(out=tmp[:, :, :, 0], in0=t0, in1=s_, op=mybir.AluOpType.mult)
                        nc.vector.tensor_tensor(out=tmp[:, :, :, 1], in0=t1, in1=c_, op=mybir.AluOpType.mult)
                        nc.vector.tensor_tensor(out=rot[:, :, :, 1], in0=tmp[:, :, :, 0], in1=tmp[:, :, :, 1], op=mybir.AluOpType.add)
                        rot2 = rot.rearrange("p h k two -> p h (k two)")
                        for h in range(HL):
                            ptr = pst.tile([128, 128], BF16, tag="ptr")
                            nc.tensor.transpose(ptr[:], rot2[:, h], ident[:])
                            nc.vector.tensor_copy(out=dstT[:, h, t * 128:(t + 1) * 128], in_=ptr[:])

                # causal attention per head, sq processed in groups of 4 tiles.
                # Produces transposed attention output attT (DL, S) so the wo
                # matmul needs no kxm transpose.
                pTbuf = pers.tile([128, NT, 512], BF16)
                for h in range(HL):
                    for g in range(NT // 4):
                        for ti in range(4):
                            tq = g * 4 + ti
                            nsk = tq + 1
                            L = nsk * 128
                            strip = strips.tile([128, S], F32, tag="strip")
                            probs = strips.tile([128, S], BF16, tag="probs")
                            nmax = stats.tile([128, 1], F32, tag="nmax")
                            rsum = stats.tile([128, 1], F32, tag="rsum")
                            rinv = stats.tile([128, 1], F32, tag="rinv")
                            lhs_q = qT[:, h, ts(tq, 128)]
                            for c0 in range(0, nsk, 4):
                                w = min(4, nsk - c0)
                                ps = pst.tile([128, 512], F32, tag="scores")
                                nc.tensor.matmul(ps[:, :w * 128], lhs_q, kT[:, h, ds(c0 * 128, w * 128)], start=True, stop=True)
                                nc.scalar.activation(strip[:, ds(c0 * 128, w * 128)], ps[:, :w * 128],
                                                     mybir.ActivationFunctionType.Copy, scale=SCALE)
                            nc.vector.tensor_tensor(out=strip[:, ds(tq * 128, 128)], in0=strip[:, ds(tq * 128, 128)],
                                                    in1=cmask[:], op=mybir.AluOpType.add)
                            nc.vector.reduce_max(nmax[:], strip[:, :L], axis=mybir.AxisListType.X)
                            nc.vector.tensor_scalar_mul(nmax[:], nmax[:], -1.0)
                            nc.scalar.activation(probs[:, :L], strip[:, :L], mybir.ActivationFunctionType.Exp,
                                                 bias=nmax[:], scale=1.0, accum_out=rsum[:])
                            nc.vector.reciprocal(rinv[:], rsum[:])
                            nc.vector.tensor_scalar_mul(probs[:, :L], probs[:, :L], rinv[:])
                            for c0 in range(0, nsk, 4):
                                w = min(4, nsk - c0)
                                ptp = pst.tile([128, 512], BF16, tag="ptrans")
                                for j in range(w):
                                    nc.tensor.transpose(ptp[:, ts(j, 128)], probs[:, ts(c0 + j, 128)], ident[:])
                                for j in range(w):
                                    nc.vector.tensor_copy(out=pTbuf[:, c0 + j, ts(ti, 128)], in_=ptp[:, ts(j, 128)])
                        # zero the not-yet-causal left slices of in-group strips
                        for ti0 in range(1, 4):
                            nc.vector.memset(pTbuf[:, g * 4 + ti0, :ti0 * 128], 0.0)
                        po = pso.tile([128, 512], F32, tag="pvout")
                        nmm = g * 4 + 4
                        for sk_t in range(nmm):
                            nc.tensor.matmul(po[:], vS[:, sk_t, ds(h * 128, 128)], pTbuf[:, sk_t, :],
                                             start=(sk_t == 0), stop=(sk_t == nmm - 1))
                        ot = work.tile([128, 512], BF16, tag="attT")
                        nc.vector.tensor_copy(out=ot[:], in_=po[:])
                        nc.sync.dma_start(out=att16[h * 128:(h + 1) * 128, g * 512:(g + 1) * 512], in_=ot[:])

            # ---- stage 4: partial y = att @ wo_c.T ----
            matmul_tile_kernel(
                tc, att16[:], wo16[:], y_t.ap(),
                transpose_kxm=False, transpose_kxn=True,
            )

    nc.compile()
    return nc


def _causal_mask():
    i = np.arange(128)
    return np.where(i[None, :] <= i[:, None], 0.0, -1e9).astype(np.float32)


def _prep_inputs(x, freqs, wq, wk, wv, wo):
    x2 = np.ascontiguousarray(x.reshape(S, D).astype(np.float32))
    f64 = freqs.astype(np.float64)
    fra = ((np.mod(f64 + np.pi, 2 * np.pi)) - np.pi).astype(np.float32)
    frb = ((np.mod(f64 + np.pi / 2 + np.pi, 2 * np.pi)) - np.pi).astype(np.float32)
    cm = _causal_mask()
    in_maps = []
    for c in range(NCORES):
        sl = slice(c * DL, (c + 1) * DL)
        in_maps.append({
            "x": x2,
            "fra": fra,
            "frb": frb,
            "wq": np.ascontiguousarray(wq[sl, :]),
            "wk": np.ascontiguousarray(wk[sl, :]),
            "wv": np.ascontiguousarray(wv[sl, :]),
            "wo": np.ascontiguousarray(wo[:, sl]),
            "cmask": cm,
        })
    return in_maps


def _run(inputs, trace=False):
    if "nc" not in _CACHE:
        _CACHE["nc"] = _build()
    nc = _CACHE["nc"]
    in_maps = _prep_inputs(**inputs)
    res = run_bass_kernel_spmd(nc, in_maps, core_ids=list(range(NCORES)), trace=trace)
    y = np.zeros((S, D), dtype=np.float64)
    for c in range(NCORES):
        y += res.results[c]["y"].astype(np.float64)
    return y.astype(np.float32).reshape(B, S, D), res.exec_time_ns


def kernel(**inputs):
    y, _ = _run(inputs, trace=False)
    return y

